# revision 11
# baseline (speedup 1.0000x reference)
"""Trainium2 Bass kernel for nn_AttentionLayer (per-pixel attention + 3x3 conv).

Problem (per batch b):
    query = W1 @ img + b1                       # [Ck=64, HW]
    scores[hw, l] = sum_k query[k, hw] v[k, l]  # [HW, L=256]
    att = softmax(scores, axis=l)
    value[c, hw] = sum_l att[hw, l] v[c, l]     # [64, HW]
    cat = [img; value]                          # [320, HW]
    out = conv3x3(cat, W2) + b2                 # [256, H, W], padding=1

Distribution: pure data-parallel, batch b -> core b (B=8, 8 cores).

Key fusions:
  * scores = img^T @ M + (b1 @ v) with M = W1^T @ v  [Cin=256, L] --
    the 1x1 conv is folded in; scores contract over Cin (full K=128 tiles).
  * conv3x3 = 9 shifted 1x1 convs (matmuls) over padded planes with row
    stride 65: col 0 of each row is zero and doubles as the right pad of
    the previous row, so each (tap, y-block) input window is a single
    CONTIGUOUS [K, (r-1)*65+64] slice (stationary operand of the matmul
    must have one free dim).  Junk output columns (x=64) are discarded in
    the PSUM->SBUF copy.
  * attention value output lands directly in padded plane 2.

Precision: img-path matmuls run in float32r (fp32 data, 11-bit mantissa,
full PE rate at free-dim >= 256); attention weights and the value plane
are bf16; softmax is f32.
"""

import numpy as np
import ml_dtypes

import concourse.bass as bass
import concourse.tile as tile
from concourse import bacc, mybir
from concourse import bass_utils

F32 = mybir.dt.float32
F32R = mybir.dt.float32r
BF16 = mybir.dt.bfloat16

B = 8
CIN = 256  # img channels
CK = 64    # query/key channels
L = 256    # attention length
COUT = 256
H = W = 64
HW = H * W          # 4096
PS = W + 1          # 65: padded row stride
PH = H + 3          # 67 rows: top pad, 64 img rows, bottom pad, overrun row
NCORES = 8

# conv y-blocks: (start_row, nrows); PSUM free dim <= 512 limits to 7 rows
BLOCKS = [(7 * i, 7) for i in range(9)] + [(63, 1)]

_CACHE = {}


def _round_f32r(a):
    """Round-to-nearest-even fp32 -> fp32r (11-bit mantissa, low 12 bits zero)."""
    u = np.ascontiguousarray(a, dtype=np.float32).view(np.uint32)
    u = (u + 0x7FF + ((u >> 12) & 1)) & np.uint32(0xFFFFF000)
    return u.view(np.float32)


def _build_nc():
    nc = bacc.Bacc("TRN2", target_bir_lowering=False, debug=False)

    img_d = nc.dram_tensor("img", (CIN, HW), F32R, kind="ExternalInput")
    v_d = nc.dram_tensor("v", (CK, L), F32R, kind="ExternalInput")
    vt_d = nc.dram_tensor("vt_bf", (L, CK), BF16, kind="ExternalInput")
    w1_d = nc.dram_tensor("w1", (CK, CIN), F32R, kind="ExternalInput")
    b1_d = nc.dram_tensor("b1p", (CK, 128), F32R, kind="ExternalInput")
    w2_d = nc.dram_tensor("w2p", (128, 18, COUT), F32R, kind="ExternalInput")
    w2v_d = nc.dram_tensor("w2v", (CK, 9, COUT), BF16, kind="ExternalInput")
    b2_d = nc.dram_tensor("b2", (COUT, 1), F32, kind="ExternalInput")
    zz_d = nc.dram_tensor("zz", (128, PS), F32R, kind="ExternalInput")
    out_d = nc.dram_tensor("out", (COUT, HW), F32, kind="ExternalOutput")

    with tile.TileContext(nc) as tc:
        with (
            tc.tile_pool(name="singles", bufs=1) as singles,
            tc.tile_pool(name="sm", bufs=3) as sm,
            tc.tile_pool(name="outp", bufs=3) as outp,
            tc.tile_pool(name="ps_s", bufs=2, space="PSUM") as ps_s,
            tc.tile_pool(name="ps_v", bufs=2, space="PSUM") as ps_v,
            tc.tile_pool(name="ps_c", bufs=2, space="PSUM") as ps_c,
        ):
            # ---- resident tensors ----
            pc0 = singles.tile([128, PH, PS], F32R)
            pc1 = singles.tile([128, PH, PS], F32R)
            pc2 = singles.tile([CK, PH, PS], BF16)
            pci = [pc0, pc1]
            imgc = singles.tile([128, 2, HW], F32R)  # contiguous img, scores lhsT
            w2sb = singles.tile([128, 18, COUT], F32R)
            w2v_sb = singles.tile([CK, 9, COUT], BF16)
            vt_sb = singles.tile([128, 2, CK], BF16)
            v_sb = singles.tile([CK, L], F32R)
            w1_sb = singles.tile([CK, CIN], F32R)
            b1_sb = singles.tile([CK, 128], F32R)
            b2_sb = singles.tile([128, 2, 1], F32)
            m_sb = singles.tile([128, 2, L], F32R)
            bias_bc = singles.tile([128, L], F32)
            attT = [
                singles.tile([128, HW], BF16, tag=f"attT{lc}", name=f"attT{lc}")
                for lc in range(2)
            ]

            # ---- input DMAs + pad-zeroing ----
            # (DVE memset on float32r is an invalid ISA encoding -- zero the
            # f32r plane pads by DMA from a zeros DRAM tensor instead)
            for p in (pc0, pc1):
                nc.sync.dma_start(p[:, 0, :], zz_d[:])         # top pad row
                nc.sync.dma_start(p[:, H + 1, :], zz_d[:])     # bottom pad row
                nc.sync.dma_start(p[:, H + 2, :], zz_d[:])     # overrun row
                nc.sync.dma_start(p[:, 1:H + 1, 0:1], zz_d[:, 0:H].rearrange("p (w o) -> p w o", o=1))
            nc.vector.memset(pc2[:, 0, :], 0.0)
            nc.vector.memset(pc2[:, H + 1, :], 0.0)
            nc.vector.memset(pc2[:, H + 2, :], 0.0)
            nc.vector.memset(pc2[:, 1:H + 1, 0:1], 0.0)
            for c in range(2):
                nc.sync.dma_start(
                    pci[c][:, 1:H + 1, 1:PS],
                    img_d[c * 128:(c + 1) * 128, :].rearrange("p (h w) -> p h w", w=W),
                )
                nc.sync.dma_start(imgc[:, c, :], img_d[c * 128:(c + 1) * 128, :])

            nc.sync.dma_start(w2sb[:], w2_d[:])
            nc.sync.dma_start(w2v_sb[:], w2v_d[:])
            nc.sync.dma_start(vt_sb[:], vt_d.rearrange("(lc p) c -> p lc c", p=128))
            nc.sync.dma_start(v_sb[:], v_d[:])
            nc.sync.dma_start(w1_sb[:], w1_d[:])
            nc.sync.dma_start(b1_sb[:], b1_d[:])
            nc.sync.dma_start(b2_sb[:], b2_d.rearrange("(t p) x -> p t x", p=128))

            # ---- M = W1^T @ v  [Cin, L], bias_row = b1^T @ v  [1, L] ----
            for cc in range(2):
                ps = ps_s.tile([128, L], F32, tag="scores", name="ps_m")
                nc.tensor.matmul(
                    ps[:], w1_sb[:, cc * 128:(cc + 1) * 128], v_sb[:],
                    start=True, stop=True,
                )
                nc.vector.tensor_copy(m_sb[:, cc, :], ps[:])
            # b1 is replicated across all 128 lhsT columns host-side, so this
            # matmul directly materializes bias_row broadcast over partitions
            psb = ps_s.tile([128, L], F32, tag="scores", name="psb")
            nc.tensor.matmul(psb[:], b1_sb[:], v_sb[:], start=True, stop=True)
            nc.vector.tensor_copy(bias_bc[:], psb[:])

            # ---- scores + softmax + transpose, per 128-pixel tile ----
            for i in range(HW // 128):
                ps = ps_s.tile([128, L], F32, tag="scores")
                for cc in range(2):
                    nc.tensor.matmul(
                        ps[:], imgc[:, cc, i * 128:(i + 1) * 128], m_sb[:, cc, :],
                        start=(cc == 0), stop=(cc == 1),
                    )
                nc.vector.tensor_add(ps[:], ps[:], bias_bc[:])
                exp_sb = sm.tile([128, L], F32, tag="exp")
                den = sm.tile([128, 1], F32, tag="den")
                nc.scalar.activation(
                    exp_sb[:], ps[:], mybir.ActivationFunctionType.Exp,
                    accum_out=den[:],
                )
                rden = sm.tile([128, 1], F32, tag="rden")
                nc.vector.reciprocal(rden[:], den[:])
                att = sm.tile([128, L], BF16, tag="att")
                nc.vector.tensor_scalar_mul(att[:], exp_sb[:], rden[:])
                for lc in range(2):
                    nc.sync.dma_start(
                        attT[lc][:, i * 128:(i + 1) * 128],
                        att[:, lc * 128:(lc + 1) * 128],
                        transpose=True,
                    )

            # ---- value = v @ att^T, written into padded plane 2 ----
            for j in range(8):
                psv = ps_v.tile([CK, 8, W], F32)
                for lc in range(2):
                    nc.tensor.matmul(
                        psv[:], vt_sb[:, lc, :], attT[lc][:, j * 512:(j + 1) * 512],
                        start=(lc == 0), stop=(lc == 1),
                    )
                nc.vector.tensor_copy(pc2[:, 1 + j * 8: 9 + j * 8, 1:PS], psv[:])

            # ---- 3x3 conv: 9 shifted matmuls x 3 channel chunks ----
            pf = [p[:].rearrange("p a b -> p (a b)") for p in (pc0, pc1, pc2)]
            for ot in range(2):
                for y0, r in BLOCKS:
                    n = (r - 1) * PS + W  # contiguous window length
                    psc = ps_c.tile([128, 7 * PS], F32)
                    k = 0
                    for tap in range(9):
                        dy, dx = tap // 3, tap % 3
                        base = (y0 + dy) * PS + dx
                        for c in range(3):
                            if c < 2:
                                lhsT = w2sb[:, tap * 2 + c, ot * 128:(ot + 1) * 128]
                            else:
                                lhsT = w2v_sb[:, tap, ot * 128:(ot + 1) * 128]
                            nc.tensor.matmul(
                                psc[:, 0:n], lhsT, pf[c][0:(128 if c < 2 else CK), base:base + n],
                                start=(k == 0), stop=(k == 26),
                            )
                            k += 1
                    outt = outp.tile([128, r, W], F32, tag="outt")
                    src = psc.rearrange("p (a b) -> p a b", b=PS)[:, 0:r, 0:W]
                    nc.scalar.activation(
                        outt[:], src, mybir.ActivationFunctionType.Identity,
                        bias=b2_sb[:, ot, :],
                    )
                    nc.sync.dma_start(
                        out_d[ot * 128:(ot + 1) * 128, y0 * W:(y0 + r) * W],
                        outt[:],
                    )

    nc.compile()
    return nc


def _prep_in_maps(img_embedding, v_embedding, W1, b1, W2, b2):
    # host-side layout prep (no math beyond dtype cast / transpose / pack)
    w2t = np.ascontiguousarray(
        W2.transpose(2, 3, 1, 0).reshape(9, CIN + CK, COUT).astype(np.float32)
    )
    w2p = np.zeros((128, 18, COUT), np.float32)
    for t in range(9):
        w2p[:, t * 2 + 0, :] = w2t[t, 0:128, :]
        w2p[:, t * 2 + 1, :] = w2t[t, 128:256, :]
    w2p = _round_f32r(w2p)
    w2v = np.ascontiguousarray(
        w2t[:, 256:320, :].transpose(1, 0, 2).astype(ml_dtypes.bfloat16)
    )
    w1f = _round_f32r(W1)
    b1p = np.repeat(np.asarray(b1, np.float32).reshape(CK, 1), 128, axis=1)
    b1p = _round_f32r(b1p)
    b2f = np.ascontiguousarray(np.asarray(b2, np.float32).reshape(COUT, 1))
    zz = np.zeros((128, PS), np.float32)

    in_maps = []
    for bb in range(B):
        img = _round_f32r(np.asarray(img_embedding[bb], np.float32).reshape(CIN, HW))
        v32 = np.asarray(v_embedding[bb], np.float32)
        v = _round_f32r(v32)
        vt = np.ascontiguousarray(v32.T.astype(ml_dtypes.bfloat16))
        in_maps.append(
            {
                "img": img,
                "v": v,
                "vt_bf": vt,
                "w1": w1f,
                "b1p": b1p,
                "w2p": w2p,
                "w2v": w2v,
                "b2": b2f,
                "zz": zz,
            }
        )
    return in_maps


def get_nc():
    if "nc" not in _CACHE:
        _CACHE["nc"] = _build_nc()
    return _CACHE["nc"]


def run_spmd(inputs, trace=False, **kwargs):
    nc = get_nc()
    in_maps = _prep_in_maps(
        inputs["img_embedding"], inputs["v_embedding"],
        inputs["W1"], inputs["b1"], inputs["W2"], inputs["b2"],
    )
    return bass_utils.run_bass_kernel_spmd(
        nc, in_maps, core_ids=list(range(NCORES)), trace=trace, **kwargs
    )


def kernel(**inputs):
    res = run_spmd(inputs)
    out = np.stack([res.results[c]["out"] for c in range(NCORES)])
    return out.reshape(B, COUT, H, W).astype(np.float32)
